# revision 30
# baseline (speedup 1.0000x reference)
"""Trainium2 Bass kernel for nn_AttentionLayerDecoder (sparse segment attention).

Math (reference, with edge_index unused):
  query[h,b,v] = context[b,:] @ Wq[h]                      # [H,B,Dv]
  u[h,n]      = (x[n,:] @ Wk[h]) . query[h,batch[n],:] / sqrt(Dv)
  a[h,n]      = segment_softmax(u) over nodes of graph batch[n]
  agg[h,b,v]  = sum_{n in b} a[h,n] * (x[n,:] @ Wv[h])
  out[b,:]    = sum_h (qc*query[h,b,:] + agg[h,b,:]) @ Wf

Key algebraic restructuring (avoids materializing keys/values [H,N,Dv]):
  qk[h,b,:]   = Wk[h] @ query[h,b,:]            (tiny, host)
  u[h,n]      = x[n,:] . qk[h,batch[n],:] / 8
  e[h,n]      = exp(u[h,n])                      (no max-subtraction needed:
                                                  u ~ N(0,1), exp never overflows)
  xe[h,b,:]   = sum_{n in b} e[h,n] * x[n,:]     (PE matmul, contraction over nodes)
  S[h,b]      = sum_{n in b} e[h,n]              (PE matmul against a ones vector)
  agg[h,b,:]  = (xe[h,b,:] @ Wv[h]) / S[h,b]     (tiny @Wv fold done on host)

Sharding: batch is sorted, so shard 16 *whole graphs* per core (no collectives).
Each graph is padded to a fixed G nodes (multiple of 128) with zero columns.
x is stored TRANSPOSED in DRAM ([feat, node] per graph) so each graph loads
with one DMA of 128 partition lines x G*2 contiguous bytes (DMA-efficient).

Device per 128-node tile: PE transpose slice -> x_nat (fp16 PSUM, grouped
4 tiles/bank) -> one DVE/ACT evacuation per group; PE u-matmul into a shared
per-graph u PSUM; per graph one ACT exp, PE xe-matmuls accumulate [8,128],
and one PE matmul v. ones gives per-(tile,head) S partials (summed on host).
Pad nodes produce exp(0)=1 in S, corrected exactly on host by subtracting the
pad count. All PE work is fp16 (inputs host-cast); accumulation is fp32.
X loads are paired two graphs per DMA, spread across the SP and Pool HWDGE
queues; xe results stream out in 4-graph chunks while compute continues.
"""

import sys

if "/opt/trn_rl_repo" not in sys.path:
    sys.path.insert(0, "/opt/trn_rl_repo")

from contextlib import ExitStack

import numpy as np

import concourse.bass as bass
import concourse.tile as tile
from concourse import bacc, masks, mybir
from concourse.bass_utils import run_bass_kernel_spmd

N_CORES = 8
H = 8          # heads
DV = 64        # head dim
DE = 124       # output embedding dim
F = 128        # node feature dim (DE + 4)
B = 128        # graphs
GPC = B // N_CORES  # graphs per core

_CACHE = {}


def _build(G, loop_r=1):
    """Build the Bass module for per-graph padded size G (multiple of 128)."""
    T = G // 128  # 128-node tiles per graph
    nc = bacc.Bacc(None, target_bir_lowering=False)
    fp16 = mybir.dt.float16
    f32 = mybir.dt.float32
    AFT = mybir.ActivationFunctionType

    XD = nc.dram_tensor("XD", [GPC, F, G], fp16, kind="ExternalInput")
    QKT = nc.dram_tensor("QKT", [F, GPC * H], fp16, kind="ExternalInput")
    XEALL = nc.dram_tensor("XEALL", [H, GPC * F], f32, kind="ExternalOutput")
    SALL = nc.dram_tensor("SALL", [T * H, GPC], f32, kind="ExternalOutput")

    with tile.TileContext(nc) as tc, ExitStack() as ctx:
        const = ctx.enter_context(tc.tile_pool(name="const", bufs=1))
        xtpool = ctx.enter_context(tc.tile_pool(name="xt", bufs=4))
        xnpool = ctx.enter_context(tc.tile_pool(name="xn", bufs=2))
        epool = ctx.enter_context(tc.tile_pool(name="e", bufs=2))
        outp = ctx.enter_context(tc.tile_pool(name="outp", bufs=1))
        ps_x = ctx.enter_context(
            tc.tile_pool(name="ps_x", bufs=4, space=bass.MemorySpace.PSUM)
        )
        ps_u = ctx.enter_context(
            tc.tile_pool(name="ps_u", bufs=2, space=bass.MemorySpace.PSUM)
        )
        ps_xe = ctx.enter_context(
            tc.tile_pool(name="ps_xe", bufs=2, space=bass.MemorySpace.PSUM)
        )

        ident = const.tile([128, 128], fp16)
        masks.make_identity(nc, ident[:])
        ones = const.tile([128, 1], fp16)
        nc.gpsimd.memset(ones[:], 1.0)
        qkt = const.tile([F, GPC * H], fp16)
        nc.gpsimd.dma_start(qkt[:], QKT[:])

        xe_all = outp.tile([H, GPC * F], f32)   # xe[h, g*128+feat]
        s_all = outp.tile([T * H, GPC], f32)    # per-tile S partials

        loop_cm = tc.For_i(0, loop_r, 1) if loop_r > 1 else None
        if loop_cm is not None:
            loop_cm.__enter__()
        xT_pair = [None]
        for g in range(GPC):
            if g % 2 == 0:
                xT_pair[0] = xtpool.tile([F, 2 * G], fp16, tag="xtg", name=f"xtg{g}")
                dma_eng = nc.sync if (g // 2) % 2 == 0 else nc.gpsimd
                if g == 0:
                    nc.sync.dma_start(xT_pair[0][:, 0:G], XD[0])
                    nc.gpsimd.dma_start(xT_pair[0][:, G : 2 * G], XD[1])
                elif g == GPC - 2:
                    nc.sync.dma_start(xT_pair[0][:, 0:G], XD[g])
                    nc.gpsimd.dma_start(xT_pair[0][:, G : 2 * G], XD[g + 1])
                else:
                    dma_eng.dma_start(
                        xT_pair[0][:].rearrange("p (g n) -> p g n", g=2),
                        XD[g : g + 2].rearrange("g f n -> f g n"),
                    )
            xT_g = xT_pair[0][:, (g % 2) * G : (g % 2 + 1) * G]
            u_ps = ps_u.tile([128, T * H], f32)
            xn_g = xnpool.tile([128, T * F], fp16)
            on_act = g % 16 in (1, 4, 7, 10, 13)
            GRP = 8 if T <= 8 else 4  # one PSUM bank holds up to 8 fp16 tiles
            for t0 in range(0, T, GRP):
                grp = min(GRP, T - t0)
                xn_ps = ps_x.tile(
                    [128, grp * F], fp16, tag="xn_ps", name=f"xnps{g}_{t0}"
                )
                for j in range(grp):
                    t = t0 + j
                    # x_nat = (xT slice)^T  (PE transpose via identity)
                    nc.tensor.transpose(
                        xn_ps[:, bass.ts(j, F)],
                        xT_g[:, bass.ts(t, 128)],
                        ident[:],
                    )
                    # u[node, head] = xT.T @ qk_g  (contraction over features)
                    nc.tensor.matmul(
                        u_ps[:, bass.ts(t, H)],
                        xT_g[:, bass.ts(t, 128)],
                        qkt[:, g * H : (g + 1) * H],
                        start=True,
                        stop=True,
                    )
                # evacuate in <=4-tile chunks (overlaps trailing transposes)
                for c0 in range(0, grp, 4):
                    cw = min(4, grp - c0)
                    srcap = xn_ps[:, c0 * F : (c0 + cw) * F]
                    dst = xn_g[:, (t0 + c0) * F : (t0 + c0 + cw) * F]
                    if on_act:
                        nc.scalar.copy(dst, srcap)
                    else:
                        nc.vector.tensor_copy(dst, srcap)
            # e = exp(u)  (PSUM f32 -> SBUF fp16)
            e_sb = epool.tile([128, T * H], fp16)
            nc.scalar.activation(e_sb[:], u_ps[:], AFT.Exp)
            # xe[head, feat] += e_tile.T @ x_nat_tile  (contraction over nodes)
            xe_ps = ps_xe.tile([H, F], f32, tag="xe", name=f"xeps{g}")
            for t in range(T):
                nc.tensor.matmul(
                    xe_ps[:],
                    e_sb[:, bass.ts(t, H)],
                    xn_g[:, bass.ts(t, F)],
                    start=(t == 0),
                    stop=(t == T - 1),
                )
            # S partials: s56[(t,h)] = sum_n e[n, (t,h)]  (pads add exp(0)=1)
            s_ps = ps_xe.tile([T * H, 1], f32, tag="xe", name=f"sps{g}")
            nc.tensor.matmul(s_ps[:], e_sb[:], ones[:], start=True, stop=True)
            nc.vector.tensor_copy(xe_all[:, g * F : (g + 1) * F], xe_ps[:])
            nc.vector.tensor_copy(s_all[:, g : g + 1], s_ps[:])
            if g % 4 == 3:
                chunk_eng = nc.gpsimd if g in (3, 11) else nc.sync
                chunk_eng.dma_start(
                    XEALL[:, (g - 3) * F : (g + 1) * F],
                    xe_all[:, (g - 3) * F : (g + 1) * F],
                )

        if loop_cm is not None:
            loop_cm.__exit__(None, None, None)
        nc.gpsimd.dma_start(SALL[:], s_all[:])

    nc.compile()
    return nc


def _get(G, loop_r=1):
    key = (G, loop_r)
    if key not in _CACHE:
        _CACHE[key] = _build(G, loop_r)
    return _CACHE[key]


def _prepare(x, batch, context, Wq, Wk):
    """Host-side shard prep. Returns (in_maps, G, query, n_pad)."""
    counts = np.bincount(batch, minlength=B).astype(np.int64)
    G = int(np.ceil(max(int(counts.max()), 1) / 128.0) * 128)
    starts = np.zeros(B + 1, np.int64)
    np.cumsum(counts, out=starts[1:])

    query = np.einsum("bc,hcv->hbv", context, Wq).astype(np.float32)  # [H,B,Dv]
    qk = np.einsum("hbv,hev->hbe", query, Wk).astype(np.float32)      # [H,B,F]
    qk8 = (qk / 8.0).astype(np.float16)

    x16 = x.astype(np.float16)

    in_maps = []
    for c in range(N_CORES):
        XDc = np.zeros((GPC, F, G), np.float16)
        QKTc = np.zeros((F, GPC * H), np.float16)
        for gi in range(GPC):
            b = c * GPC + gi
            n0, n1 = int(starts[b]), int(starts[b + 1])
            XDc[gi, :, 0 : n1 - n0] = x16[n0:n1].T
            QKTc[:, gi * H : (gi + 1) * H] = qk8[:, b, :].T
        in_maps.append({"XD": XDc, "QKT": QKTc})
    n_pad = (G - counts).astype(np.float32)  # [B]
    return in_maps, G, query, n_pad


def kernel(**inputs):
    x = np.asarray(inputs["x"], np.float32)
    batch = np.asarray(inputs["batch"]).astype(np.int64)
    context = np.asarray(inputs["context"], np.float32)
    Wq = np.asarray(inputs["Wq"], np.float32)
    Wk = np.asarray(inputs["Wk"], np.float32)
    Wv = np.asarray(inputs["Wv"], np.float32)
    qc = float(np.asarray(inputs["query_coef"]).reshape(-1)[0])
    Wf = np.asarray(inputs["Wf"], np.float32)

    in_maps, G, query, n_pad = _prepare(x, batch, context, Wq, Wk)
    T = G // 128

    nc = _get(G)
    res = run_bass_kernel_spmd(nc, in_maps, core_ids=list(range(N_CORES)))

    XE = np.zeros((H, B, F), np.float32)
    S = np.zeros((H, B), np.float32)
    for c in range(N_CORES):
        xe = res.results[c]["XEALL"].reshape(H, GPC, F)
        XE[:, c * GPC : (c + 1) * GPC, :] = xe
        s56 = res.results[c]["SALL"].reshape(T, H, GPC)
        S[:, c * GPC : (c + 1) * GPC] = s56.sum(axis=0)

    S = S - n_pad[None, :]  # pad rows contributed exp(0)*1 each to S
    Y = np.einsum("hbe,hev->hbv", XE, Wv.astype(np.float32))
    agg = Y / (S[..., None] + 1e-16)
    hbv = qc * query + agg
    out = np.einsum("hbv,ve->be", hbv, Wf)
    return out.astype(np.float32)
